# revision 2
# baseline (speedup 1.0000x reference)
"""Category-specific linear (MoE routing) kernel for 8 Trainium2 cores.

out[b] = x[b] @ W[cat_ids[b]] + b[cat_ids[b]]
  x: [256, 64, 1024] f32, cat_ids: [256] int, W: [64, 1024, 1024] f32,
  b: [64, 1024] f32 -> out: [256, 64, 1024] f32

Strategy (memory-regime): group samples by category so each expert's
weight block is streamed from HBM once per chip. Categories (chunked to at
most T_MAX samples) are dealt by size-rank across the 8 cores, giving every
core the same static "template" of group sizes — one SPMD program. The only
per-core dynamic state is which category each group uses, passed as an
int32 index tile consumed by indirect-DMA gathers of W rows on device.

All device-side tensors are bf16 (tolerance is 2e-2; bf16 end-to-end error
is ~2e-3), halving HBM traffic vs f32. PSUM accumulation stays f32.
x is marshalled on host into a per-group packed transposed layout so each
group's x load is a single fully-contiguous-per-partition DMA.
"""
import math
from functools import lru_cache

import numpy as np
import ml_dtypes

import concourse.bass as bass
import concourse.mybir as mybir
import concourse.tile as tile
from concourse import bacc
from concourse.bass_utils import run_bass_kernel_spmd

# Problem shapes (hardcoded per task spec)
B = 256
S = 64
D = 1024  # input dim (contraction)
H = 1024  # hidden dim
C = 64    # num categories
N_CORES = 8
T_MAX = 8     # max sample slots per group (one weight load per group)
P = 128       # partitions
KC = D // P   # 8 contraction chunks
NT = H // 512  # 2 psum n-tiles

_f32 = mybir.dt.float32
_bf16 = mybir.dt.bfloat16
_np_bf16 = ml_dtypes.bfloat16


def plan_routing(cat_ids):
    """Split categories into <=T_MAX-sample chunks, deal chunks by size rank
    across cores. Returns (template, per_core_groups) where
    per_core_groups[c] is a list of (cat, [sample_indices]) aligned to
    template (padded with dummy (0, []) entries)."""
    cat_ids = np.asarray(cat_ids).astype(np.int64)
    by_cat = {}
    for i, c in enumerate(cat_ids.tolist()):
        by_cat.setdefault(c, []).append(i)
    items = []  # (size, cat, samples)
    for c, samp in by_cat.items():
        for off in range(0, len(samp), T_MAX):
            chunk = samp[off:off + T_MAX]
            items.append((len(chunk), c, chunk))
    items.sort(key=lambda t: -t[0])
    G = max(1, math.ceil(len(items) / N_CORES))
    per_core = [[] for _ in range(N_CORES)]
    for rank, it in enumerate(items):
        per_core[rank % N_CORES].append(it)
    template = []
    for g in range(G):
        template.append(max((core[g][0] for core in per_core if len(core) > g),
                            default=1))
    per_core_groups = []
    for core in per_core:
        groups = [(cat, samp) for (_, cat, samp) in core]
        while len(groups) < G:
            groups.append((0, []))
        per_core_groups.append(groups)
    return tuple(template), per_core_groups


def build_kernel(template, wp_bufs=3, xp_bufs=2, op_bufs=4, pp_bufs=8,
                 loop_repeat=None, with_bias=True):
    """Build the SPMD Bass kernel for a given group-size template.

    loop_repeat: run the body in a hardware For_i loop — timing harness use
    only; grading path uses default (None).
    """
    G = len(template)
    R = 64 * sum(template)          # padded rows per core
    m_max = 64 * max(template)

    nc = bacc.Bacc("TRN2", target_bir_lowering=False, debug=False)
    xP = nc.dram_tensor("xP", [P, KC * R], _bf16, kind="ExternalInput")
    W2 = nc.dram_tensor("W2", [C * D, H], _bf16, kind="ExternalInput")
    widx = nc.dram_tensor("widx", [P, G * KC], mybir.dt.int32, kind="ExternalInput")
    biasg = nc.dram_tensor("biasg", [1, G * H + P], _bf16, kind="ExternalInput")
    out = nc.dram_tensor("out", [R, H], _bf16, kind="ExternalOutput")

    with tile.TileContext(nc) as tc:
        with tc.tile_pool(name="wp", bufs=wp_bufs) as wp, \
             tc.tile_pool(name="xp", bufs=xp_bufs) as xp, \
             tc.tile_pool(name="op", bufs=op_bufs) as op, \
             tc.tile_pool(name="cst", bufs=1) as cst, \
             tc.tile_pool(name="pp", bufs=pp_bufs, space="PSUM") as pp:

            idx_t = cst.tile([P, G * KC], mybir.dt.int32)
            nc.sync.dma_start(out=idx_t[:], in_=widx.ap())
            bias_t = cst.tile([1, G * H + P], _bf16)
            nc.sync.dma_start(out=bias_t[:], in_=biasg.ap())
            ones_t = bias_t[:, G * H:G * H + P]

            def load_w(g, w_t):
                for kc in range(KC):
                    nc.gpsimd.indirect_dma_start(
                        out=w_t[:, kc * H:(kc + 1) * H],
                        out_offset=None,
                        in_=W2.ap(),
                        in_offset=bass.IndirectOffsetOnAxis(
                            ap=idx_t[:, g * KC + kc:g * KC + kc + 1], axis=0),
                    )

            def load_x(g, m_off, Mg, x_t):
                nc.sync.dma_start(
                    out=x_t[:, :KC * Mg],
                    in_=xP.ap()[:, KC * m_off:KC * (m_off + Mg)],
                )

            def body():
                m_off = 0
                for g in range(G):
                    Tg = template[g]
                    Mg = 64 * Tg
                    w_t = wp.tile([P, KC * H], _bf16, tag="w")
                    load_w(g, w_t)
                    x_t = xp.tile([P, KC * m_max], _bf16, tag="x")
                    load_x(g, m_off, Mg, x_t)
                    for mt in range(math.ceil(Mg / P)):
                        rows = min(P, Mg - mt * P)
                        o_t = op.tile([P, H], _bf16, tag="o")
                        for n in range(NT):
                            ps = pp.tile([P, 512], _f32, space="PSUM")
                            if with_bias:
                                nc.tensor.matmul(
                                    out=ps[:rows, :],
                                    lhsT=ones_t[:1, :rows],
                                    rhs=bias_t[:1, g * H + n * 512:g * H + (n + 1) * 512],
                                    start=True, stop=False,
                                )
                            for kc in range(KC):
                                nc.tensor.matmul(
                                    out=ps[:rows, :],
                                    lhsT=x_t[:, kc * Mg + mt * P:kc * Mg + mt * P + rows],
                                    rhs=w_t[:, kc * H + n * 512:kc * H + (n + 1) * 512],
                                    start=(kc == 0 and not with_bias),
                                    stop=(kc == KC - 1),
                                )
                            nc.vector.tensor_copy(
                                out=o_t[:rows, n * 512:(n + 1) * 512],
                                in_=ps[:rows, :],
                            )
                        nc.sync.dma_start(
                            out=out.ap()[m_off + mt * P:m_off + mt * P + rows, :],
                            in_=o_t[:rows, :],
                        )
                    m_off += Mg

            if loop_repeat is not None:
                with tc.For_i(0, loop_repeat, 1):
                    body()
            else:
                body()
    nc.compile()
    return nc


@lru_cache(maxsize=8)
def _kernel_for(template, loop_repeat=None, with_bias=True):
    return build_kernel(template, loop_repeat=loop_repeat, with_bias=with_bias)


def make_inputs(x, cat_ids, W, b, template, per_core_groups):
    """Build per-core input maps (host-side shard/marshal)."""
    G = len(template)
    R = 64 * sum(template)
    W2 = np.ascontiguousarray(W.reshape(C * D, H)).astype(_np_bf16)
    slot_off = np.concatenate([[0], np.cumsum(template)]).astype(np.int64)
    in_maps = []
    placements = []  # per core: list of (row_start, sample_index)
    for core in range(N_CORES):
        xPc = np.zeros((P, KC * R), dtype=_np_bf16)
        widx = np.zeros((P, G * KC), dtype=np.int32)
        biasg = np.zeros((1, G * H + P), dtype=_np_bf16)
        biasg[0, G * H:] = 1.0
        place = []
        for g, (cat, samp) in enumerate(per_core_groups[core]):
            widx[:, g * KC:(g + 1) * KC] = (
                cat * D + np.arange(KC)[None, :] * P + np.arange(P)[:, None]
            )
            biasg[0, g * H:(g + 1) * H] = b[cat].astype(_np_bf16)
            Mg = 64 * template[g]
            m0 = int(slot_off[g]) * 64
            if samp:
                xs = np.asarray(x)[samp]           # [n, 64, D]
                n = xs.shape[0]
                # [D, n*64] -> [KC, P, n*64] -> [P, KC, n*64]
                arr = np.zeros((P, KC, Mg), dtype=_np_bf16)
                arr[:, :, :n * 64] = (
                    xs.reshape(n * 64, D).T.reshape(KC, P, n * 64)
                    .transpose(1, 0, 2).astype(_np_bf16)
                )
                xPc[:, KC * m0:KC * (m0 + Mg)] = arr.reshape(P, KC * Mg)
                for j, bi in enumerate(samp):
                    place.append((m0 + j * 64, bi))
        in_maps.append({"xP": xPc, "W2": W2, "widx": widx, "biasg": biasg})
        placements.append(place)
    return in_maps, placements


def kernel(x, cat_ids, W, b):
    x = np.asarray(x, dtype=np.float32)
    W = np.asarray(W, dtype=np.float32)
    b = np.asarray(b, dtype=np.float32)
    template, per_core_groups = plan_routing(cat_ids)
    # all-zero bias (the spec's fill) needs no bias matmuls on device
    nc = _kernel_for(template, with_bias=bool(np.any(b)))
    in_maps, placements = make_inputs(x, cat_ids, W, b, template, per_core_groups)
    res = run_bass_kernel_spmd(nc, in_maps, core_ids=list(range(N_CORES)))
    out = np.empty((B, S, H), dtype=np.float32)
    for core in range(N_CORES):
        oc = np.asarray(res.results[core]["out"], dtype=np.float32)
        for row0, bi in placements[core]:
            out[bi] = oc[row0:row0 + 64, :]
    return out


# revision 32
# speedup vs baseline: 1.2072x; 1.2072x over previous
"""Category-specific linear (MoE routing) kernel for 8 Trainium2 cores.

out[b] = x[b] @ W[cat_ids[b]] + b[cat_ids[b]]
  x: [256, 64, 1024] f32, cat_ids: [256] int, W: [64, 1024, 1024] f32,
  b: [64, 1024] f32 -> out: [256, 64, 1024] f32

Strategy (memory-regime): group samples by category so each expert's
weight block is streamed from HBM once per chip. Categories (chunked to at
most T_MAX samples) are dealt by size-rank across the 8 cores, giving every
core the same static "template" of group sizes — one SPMD program. The only
per-core dynamic state is which category each group uses, passed as an
int32 index tile consumed by indirect-DMA gathers of W rows on device.

All device-side tensors are bf16 (tolerance is 2e-2; bf16 end-to-end error
is ~2e-3), halving HBM traffic vs f32. PSUM accumulation stays f32.
x is marshalled on host into a per-group packed transposed layout so each
group's x load is a single fully-contiguous-per-partition DMA.
"""
import math
from functools import lru_cache

import numpy as np
import ml_dtypes

import concourse.bass as bass
import concourse.mybir as mybir
import concourse.tile as tile
from concourse import bacc
from concourse.bass_utils import run_bass_kernel_spmd

# Problem shapes (hardcoded per task spec)
B = 256
S = 64
D = 1024  # input dim (contraction)
H = 1024  # hidden dim
C = 64    # num categories
N_CORES = 8
T_MAX = 8     # max sample slots per group (one weight load per group)
P = 128       # partitions
KC = D // P   # 8 contraction chunks
NT = H // 512  # 2 psum n-tiles

_f32 = mybir.dt.float32
_bf16 = mybir.dt.bfloat16
_np_bf16 = ml_dtypes.bfloat16


def _kept_slots(n, t):
    """x/out slots actually transferred for a chunk of n real samples in a
    template-t slot when fully-padded m-tiles are cond-skipped."""
    return min(2 * math.ceil(n / 2), t)


def plan_routing(cat_ids):
    """Split categories into <=T_MAX-sample chunks, deal chunks by size rank
    across cores balancing per-core kept-slot totals (the DMA bytes that
    survive cond-skipping). Returns (template, per_core_groups) where
    per_core_groups[c] is a list of (cat, [sample_indices]) aligned to
    template (padded with dummy (0, []) entries)."""
    cat_ids = np.asarray(cat_ids).astype(np.int64)
    by_cat = {}
    for i, c in enumerate(cat_ids.tolist()):
        by_cat.setdefault(c, []).append(i)
    items = []  # (size, cat, samples)
    for c, samp in by_cat.items():
        for off in range(0, len(samp), T_MAX):
            chunk = samp[off:off + T_MAX]
            items.append((len(chunk), c, chunk))
    items.sort(key=lambda t: -t[0])
    G = max(1, math.ceil(len(items) / N_CORES))
    while len(items) < G * N_CORES:
        items.append((0, 0, []))
    template = tuple(items[g * N_CORES][0] if items[g * N_CORES][0] > 0 else 1
                     for g in range(G))
    # Per-rank LPT on kept slots, then swap refinement to minimize the max.
    per_core = [[None] * G for _ in range(N_CORES)]
    load = [0] * N_CORES
    for g in range(G):
        rank = items[g * N_CORES:(g + 1) * N_CORES]
        order = sorted(range(N_CORES), key=lambda c: load[c])
        for i, it in enumerate(rank):
            c = order[i]
            per_core[c][g] = it
            load[c] += _kept_slots(it[0], template[g])
    for _ in range(4):  # swap refinement
        improved = False
        hi = max(range(N_CORES), key=lambda c: load[c])
        for g in range(G):
            for c2 in range(N_CORES):
                if c2 == hi:
                    continue
                a, bch = per_core[hi][g], per_core[c2][g]
                da = _kept_slots(a[0], template[g])
                db = _kept_slots(bch[0], template[g])
                if max(load[hi] - da + db, load[c2] - db + da) < max(
                        load[hi], load[c2]):
                    per_core[hi][g], per_core[c2][g] = bch, a
                    load[hi] += db - da
                    load[c2] += da - db
                    improved = True
                    hi = max(range(N_CORES), key=lambda c: load[c])
        if not improved:
            break
    per_core_groups = [[(cat, samp) for (_, cat, samp) in core]
                       for core in per_core]
    return template, per_core_groups


def _tiles_of(template):
    """Static m-tile metadata: (group, mt, global_row0, rows, x_col_off)."""
    tiles = []
    m_off = 0
    xcol = 0
    for g, t in enumerate(template):
        Mg = 64 * t
        for mt in range(math.ceil(Mg / P)):
            rows = min(P, Mg - mt * P)
            tiles.append((g, mt, m_off + mt * P, rows, xcol))
            xcol += KC * rows
        m_off += Mg
    return tiles


def build_kernel(template, wp_bufs=3, xp_bufs=2, op_bufs=4, pp_bufs=8,
                 loop_repeat=None, with_bias=True, w_mode="indirect",
                 w_engines=("gpsimd", "vector"), x_engine="sync",
                 out_engine="sync", skip=False, copy_engine="vector",
                 x_mode="group"):
    """Build the SPMD Bass kernel for a given group-size template.

    loop_repeat: run the body in a hardware For_i loop — timing harness use
    only; grading path uses default (None).
    w_mode "packed": host pre-gathers each core's per-group weights into a
    partition-contiguous [P, G*KC*H] buffer so every W load is one large
    contiguous-per-partition DMA, split across the w_engines queues.
    w_mode "indirect": on-device gather of W rows from the full replicated
    table via an int32 index tile.
    """
    G = len(template)
    R = 64 * sum(template)          # padded rows per core
    m_max = 64 * max(template)
    tiles = _tiles_of(template)
    NTt = len(tiles)

    n_static = int(w_mode[-1]) if w_mode.startswith("hybrid") else 0

    nc = bacc.Bacc("TRN2", target_bir_lowering=False, debug=False)
    xP = nc.dram_tensor("xP", [P, KC * R], _bf16, kind="ExternalInput")
    if w_mode == "packed" or n_static:
        Wp = nc.dram_tensor("Wp", [P, G * KC * H], _bf16, kind="ExternalInput")
    if w_mode != "packed":
        W2 = nc.dram_tensor("W2", [C * D, H], _bf16, kind="ExternalInput")
        widx = nc.dram_tensor("widx", [P, G * KC], mybir.dt.int32,
                              kind="ExternalInput")
    if skip:
        keepf = nc.dram_tensor("keepf", [1, NTt], mybir.dt.int32,
                               kind="ExternalInput")
    biasg = nc.dram_tensor("biasg", [1, G * H + P], _bf16, kind="ExternalInput")
    out = nc.dram_tensor("out", [R, H], _bf16, kind="ExternalOutput")

    with tile.TileContext(nc) as tc:
        with tc.tile_pool(name="wp", bufs=wp_bufs) as wp, \
             tc.tile_pool(name="xp", bufs=xp_bufs) as xp, \
             tc.tile_pool(name="op", bufs=op_bufs) as op, \
             tc.tile_pool(name="cst", bufs=1) as cst, \
             tc.tile_pool(name="pp", bufs=pp_bufs, space="PSUM") as pp:

            if w_mode != "packed":
                idx_t = cst.tile([P, G * KC], mybir.dt.int32)
                nc.sync.dma_start(out=idx_t[:], in_=widx.ap())
            bias_t = cst.tile([1, G * H + P], _bf16)
            nc.sync.dma_start(out=bias_t[:], in_=biasg.ap())
            ones_t = bias_t[:, G * H:G * H + P]
            keep_vals = None
            if skip:
                keep_t = cst.tile([1, NTt], mybir.dt.int32)
                nc.sync.dma_start(out=keep_t[:], in_=keepf.ap())
                cond_engines = []
                for e in ({"split": ("sync", "scalar"),
                           "alt": ("sync", "scalar")}.get(x_engine, (x_engine,))
                          + {"alt": ("sync", "scalar")}.get(out_engine,
                                                            (out_engine,))):
                    et = getattr(nc, e).engine
                    if et not in cond_engines:
                        cond_engines.append(et)
                _, keep_vals = nc.values_load_multi_w_load_instructions(
                    keep_t[0:1, :], engines=cond_engines,
                    min_val=0, max_val=1, skip_runtime_bounds_check=True)

            w_engs = [getattr(nc, e) for e in w_engines]
            x_engs = ([nc.sync, nc.scalar] if x_engine == "split"
                      else [getattr(nc, x_engine)])
            out_engs = ([nc.sync, nc.scalar] if out_engine == "alt"
                        else [getattr(nc, out_engine)])

            def load_w(g, w_t):
                if w_mode == "packed":
                    ne = len(w_engs)
                    per = KC // ne
                    for i, eng in enumerate(w_engs):
                        lo, hi = i * per, (i + 1) * per if i < ne - 1 else KC
                        eng.dma_start(
                            out=w_t[:, lo * H:hi * H],
                            in_=Wp.ap()[:, g * KC * H + lo * H:
                                        g * KC * H + hi * H],
                        )
                    return
                st_engs = [nc.sync, nc.scalar]
                for kc in range(KC):
                    if kc >= KC - n_static:
                        st_engs[kc % 2].dma_start(
                            out=w_t[:, kc * H:(kc + 1) * H],
                            in_=Wp.ap()[:, (g * KC + kc) * H:
                                        (g * KC + kc + 1) * H],
                        )
                    else:
                        nc.gpsimd.indirect_dma_start(
                            out=w_t[:, kc * H:(kc + 1) * H],
                            out_offset=None,
                            in_=W2.ap(),
                            in_offset=bass.IndirectOffsetOnAxis(
                                ap=idx_t[:, g * KC + kc:g * KC + kc + 1], axis=0),
                        )

            def load_x(g, m_off, Mg, x_t):
                ne = len(x_engs)
                half = (KC * Mg) // ne
                for i, eng in enumerate(x_engs):
                    lo = i * half
                    hi = (i + 1) * half if i < ne - 1 else KC * Mg
                    eng.dma_start(
                        out=x_t[:, lo:hi],
                        in_=xP.ap()[:, KC * m_off + lo:KC * m_off + hi],
                    )

            cp_engs = ([nc.vector, nc.scalar] if copy_engine == "alt"
                       else [getattr(nc, copy_engine)])

            def mm_tile(g, rows, o_t, lhsT_of):
                for n in range(NT):
                    ps = pp.tile([P, 512], _f32, space="PSUM")
                    if with_bias:
                        nc.tensor.matmul(
                            out=ps[:rows, :],
                            lhsT=ones_t[:1, :rows],
                            rhs=bias_t[:1, g * H + n * 512:g * H + (n + 1) * 512],
                            start=True, stop=False,
                        )
                    for kc in range(KC):
                        nc.tensor.matmul(
                            out=ps[:rows, :],
                            lhsT=lhsT_of(kc),
                            rhs=w_t_cur[0][:, kc * H + n * 512:kc * H + (n + 1) * 512],
                            start=(kc == 0 and not with_bias),
                            stop=(kc == KC - 1),
                        )
                    cp_engs[n % len(cp_engs)].tensor_copy(
                        out=o_t[:rows, n * 512:(n + 1) * 512],
                        in_=ps[:rows, :],
                    )

            w_t_cur = [None]

            def body():
                if skip or x_mode == "mtile":
                    last_g = -1
                    for ti, (g, mt, row0, rows, xcol) in enumerate(tiles):
                        if g != last_g:
                            w_t_cur[0] = wp.tile([P, KC * H], _bf16, tag="w",
                                                 name="w_t")
                            load_w(g, w_t_cur[0])
                            last_g = g
                        cond = keep_vals[ti] if skip else None
                        x_t = xp.tile([P, KC * P], _bf16, tag="x")
                        x_engs[ti % len(x_engs)].dma_start(
                            out=x_t[:, :KC * rows],
                            in_=xP.ap()[:, xcol:xcol + KC * rows],
                            cond=cond,
                        )
                        o_t = op.tile([P, H], _bf16, tag="o")
                        mm_tile(g, rows, o_t,
                                lambda kc: x_t[:, kc * rows:(kc + 1) * rows])
                        out_engs[ti % len(out_engs)].dma_start(
                            out=out.ap()[row0:row0 + rows, :],
                            in_=o_t[:rows, :],
                            cond=cond,
                        )
                else:
                    m_off = 0
                    ti = 0
                    for g in range(G):
                        Tg = template[g]
                        Mg = 64 * Tg
                        w_t_cur[0] = wp.tile([P, KC * H], _bf16, tag="w",
                                             name="w_t")
                        load_w(g, w_t_cur[0])
                        x_t = xp.tile([P, KC * m_max], _bf16, tag="x")
                        load_x(g, m_off, Mg, x_t)
                        for mt in range(math.ceil(Mg / P)):
                            rows = min(P, Mg - mt * P)
                            o_t = op.tile([P, H], _bf16, tag="o")
                            mm_tile(g, rows, o_t,
                                    lambda kc: x_t[:, kc * Mg + mt * P:
                                                   kc * Mg + mt * P + rows])
                            out_engs[ti % len(out_engs)].dma_start(
                                out=out.ap()[m_off + mt * P:
                                             m_off + mt * P + rows, :],
                                in_=o_t[:rows, :],
                            )
                            ti += 1
                        m_off += Mg

            if loop_repeat is not None:
                with tc.For_i(0, loop_repeat, 1):
                    body()
            else:
                body()
    nc.compile()
    return nc


def _kernel_for(template, loop_repeat=None, with_bias=True, **kw):
    merged = {**CONFIG, **kw}
    return _kernel_for_cached(template, loop_repeat, with_bias,
                              tuple(sorted(merged.items())))


@lru_cache(maxsize=32)
def _kernel_for_cached(template, loop_repeat, with_bias, kw_items):
    return build_kernel(template, loop_repeat=loop_repeat, with_bias=with_bias,
                        **dict(kw_items))


def make_inputs(x, cat_ids, W, b, template, per_core_groups, w_mode=None,
                skip=None, x_mode=None):
    if w_mode is None:
        w_mode = CONFIG["w_mode"]
    if skip is None:
        skip = CONFIG["skip"]
    if x_mode is None:
        x_mode = CONFIG.get("x_mode", "group")
    tile_layout = skip or x_mode == "mtile"
    """Build per-core input maps (host-side shard/marshal)."""
    G = len(template)
    R = 64 * sum(template)
    tiles = _tiles_of(template)
    need_packed = w_mode == "packed" or w_mode.startswith("hybrid")
    need_idx = w_mode != "packed"
    # [C, KC, P, H] bf16 view of W for per-core packing / row indexing
    Wb = W.reshape(C, KC, P, H).astype(_np_bf16)
    if need_idx:
        W2 = np.ascontiguousarray(Wb.reshape(C * D, H))
    slot_off = np.concatenate([[0], np.cumsum(template)]).astype(np.int64)
    x = np.asarray(x)
    in_maps = []
    placements = []  # per core: list of (row_start, sample_index)
    for core in range(N_CORES):
        xPc = np.zeros((P, KC * R), dtype=_np_bf16)
        biasg = np.zeros((1, G * H + P), dtype=_np_bf16)
        biasg[0, G * H:] = 1.0
        cats = [cat for (cat, _) in per_core_groups[core]]
        if need_packed:
            # [G, KC, P, H] -> [P, G, KC, H] partition-contiguous layout
            Wpc = np.ascontiguousarray(
                Wb[cats].transpose(2, 0, 1, 3)).reshape(P, G * KC * H)
        if need_idx:
            widx = np.zeros((P, G * KC), dtype=np.int32)
        place = []
        xg = {}  # group -> [P, KC, n*64] transposed real x
        for g, (cat, samp) in enumerate(per_core_groups[core]):
            if need_idx:
                widx[:, g * KC:(g + 1) * KC] = (
                    cat * D + np.arange(KC)[None, :] * P
                    + np.arange(P)[:, None]
                )
            biasg[0, g * H:(g + 1) * H] = b[cat].astype(_np_bf16)
            m0 = int(slot_off[g]) * 64
            if samp:
                xs = x[samp]                       # [n, 64, D]
                n = xs.shape[0]
                xg[g] = (xs.reshape(n * 64, D).T.reshape(KC, P, n * 64)
                         .transpose(1, 0, 2).astype(_np_bf16))
                for j, bi in enumerate(samp):
                    place.append((m0 + j * 64, bi))
        if tile_layout:
            keep = np.zeros((1, len(tiles)), dtype=np.int32)
            for ti, (g, mt, row0, rows, xcol) in enumerate(tiles):
                if g not in xg:
                    continue
                nreal = xg[g].shape[2]
                r0 = mt * P
                rr = min(rows, nreal - r0)
                if rr <= 0:
                    continue
                keep[0, ti] = 1
                blk = np.zeros((P, KC, rows), dtype=_np_bf16)
                blk[:, :, :rr] = xg[g][:, :, r0:r0 + rr]
                xPc[:, xcol:xcol + KC * rows] = blk.reshape(P, KC * rows)
        else:
            for g, t in enumerate(template):
                if g not in xg:
                    continue
                Mg = 64 * t
                m0 = int(slot_off[g]) * 64
                n64 = xg[g].shape[2]
                arr = np.zeros((P, KC, Mg), dtype=_np_bf16)
                arr[:, :, :n64] = xg[g]
                xPc[:, KC * m0:KC * (m0 + Mg)] = arr.reshape(P, KC * Mg)
        m = {"xP": xPc, "biasg": biasg}
        if need_packed:
            m["Wp"] = Wpc
        if need_idx:
            m["W2"] = W2
            m["widx"] = widx
        if skip:
            m["keepf"] = keep
        in_maps.append(m)
        placements.append(place)
    return in_maps, placements


# Device-kernel configuration used by kernel(); sweep-validated.
CONFIG = dict(w_mode="indirect", skip=False, x_engine="split",
              out_engine="sync", wp_bufs=4)


def kernel(x, cat_ids, W, b):
    x = np.asarray(x, dtype=np.float32)
    W = np.asarray(W, dtype=np.float32)
    b = np.asarray(b, dtype=np.float32)
    template, per_core_groups = plan_routing(cat_ids)
    # all-zero bias (the spec's fill) needs no bias matmuls on device
    nc = _kernel_for(template, with_bias=bool(np.any(b)), **CONFIG)
    in_maps, placements = make_inputs(x, cat_ids, W, b, template,
                                      per_core_groups,
                                      w_mode=CONFIG["w_mode"],
                                      skip=CONFIG["skip"])
    res = run_bass_kernel_spmd(nc, in_maps, core_ids=list(range(N_CORES)))
    out = np.empty((B, S, H), dtype=np.float32)
    for core in range(N_CORES):
        oc = np.asarray(res.results[core]["out"], dtype=np.float32)
        for row0, bi in placements[core]:
            out[bi] = oc[row0:row0 + 64, :]
    return out
